# revision 6
# baseline (speedup 1.0000x reference)
"""Diag-scale kernel: out = input * W (W broadcast along rows).

input: (16384, 4096) f32, W: (4096,) f32. Data-parallel over 8 NeuronCores:
each core gets 2048 rows; W is replicated (pre-broadcast to [128, R*D] on
host so no on-chip partition broadcast is needed).
"""

import os
import numpy as np

import concourse.bass as bass
import concourse.bacc as bacc
import concourse.mybir as mybir
from concourse.tile import TileContext
from concourse.bass_utils import run_bass_kernel_spmd

N = 16384
D = 4096
NCORES = 8
ROWS = N // NCORES          # 2048 rows per core
P = 128                     # SBUF partitions
R = 2                       # rows of D per partition per tile
FREE = R * D                # 8192 f32 = 32KB per partition per tile
NTILES = ROWS // (P * R)    # 8 tiles of [128, 8192] (4 MiB each)
IO_BUFS = 3

last_exec_time_ns = None
last_trace_dir = None
_built_nc = None


def _build():
    nc = bacc.Bacc(None, target_bir_lowering=False, debug=False)
    inp = nc.declare_dram_parameter("input", [ROWS, D], mybir.dt.float32, isOutput=False)
    w = nc.declare_dram_parameter("w", [P, FREE], mybir.dt.float32, isOutput=False)
    out = nc.declare_dram_parameter("out", [ROWS, D], mybir.dt.float32, isOutput=True)

    # row index = (n*128 + p)*R + r  ->  tile n, partition p holds R
    # consecutive rows; each tile is one contiguous 4 MiB DRAM block.
    inp_t = inp[:, :].rearrange("(n p r) d -> n p (r d)", p=P, r=R)
    out_t = out[:, :].rearrange("(n p r) d -> n p (r d)", p=P, r=R)

    with TileContext(nc) as tc:
        with (
            tc.tile_pool(name="wpool", bufs=1) as wpool,
            tc.tile_pool(name="io", bufs=IO_BUFS) as io,
        ):
            wt = wpool.tile([P, FREE], mybir.dt.float32)
            nc.sync.dma_start(out=wt[:], in_=w[:, :])
            for i in range(NTILES):
                t = io.tile([P, FREE], mybir.dt.float32)
                nc.sync.dma_start(out=t[:], in_=inp_t[i])
                nc.vector.tensor_mul(out=t[:], in0=t[:], in1=wt[:])
                nc.sync.dma_start(out=out_t[i], in_=t[:])
    nc.compile()
    return nc


def kernel(input, W):
    global last_exec_time_ns, _built_nc
    input = np.ascontiguousarray(np.asarray(input, dtype=np.float32))
    W = np.asarray(W, dtype=np.float32).reshape(D)

    if _built_nc is None:
        _built_nc = _build()
    nc = _built_nc

    # each free-dim element (r*D + d) multiplies by W[d]; same for all partitions
    w_rep = np.ascontiguousarray(np.broadcast_to(np.tile(W, R), (P, FREE)))
    shards = input.reshape(NCORES, ROWS, D)
    in_maps = [{"input": shards[c], "w": w_rep} for c in range(NCORES)]

    global last_trace_dir
    trace = os.environ.get("KERNEL_TRACE", "0") == "1"
    kwargs = {}
    if trace:
        import tempfile

        last_trace_dir = tempfile.mkdtemp(prefix="diag_trace_")
        kwargs = {"trace": True, "tmpdir": last_trace_dir}
    res = run_bass_kernel_spmd(nc, in_maps, core_ids=list(range(NCORES)), **kwargs)
    last_exec_time_ns = res.exec_time_ns

    out = np.concatenate([res.results[c]["out"] for c in range(NCORES)], axis=0)
    return out


# revision 9
# speedup vs baseline: 1.3044x; 1.3044x over previous
"""Diag-scale kernel: out = input * W (W broadcast along rows).

input: (16384, 4096) f32, W: (4096,) f32. Data-parallel over 8 NeuronCores:
each core gets 2048 rows; W is replicated (pre-broadcast to [128, R*D] on
host so no on-chip partition broadcast is needed).
"""

import os
import numpy as np

import concourse.bass as bass
import concourse.bacc as bacc
import concourse.mybir as mybir
from concourse.tile import TileContext
from concourse.bass_utils import run_bass_kernel_spmd

N = 16384
D = 4096
NCORES = 8
ROWS = N // NCORES          # 2048 rows per core
P = 128                     # SBUF partitions
R = 2                       # rows of D per partition per tile
FREE = R * D                # 8192 f32 = 32KB per partition per tile
NTILES = ROWS // (P * R)    # 8 tiles of [128, 8192] (4 MiB each)
IO_BUFS = 5

last_exec_time_ns = None
last_trace_dir = None
_built_nc = None


def _build():
    nc = bacc.Bacc(None, target_bir_lowering=False, debug=False)
    inp = nc.declare_dram_parameter("input", [ROWS, D], mybir.dt.float32, isOutput=False)
    w = nc.declare_dram_parameter("w", [P, D], mybir.dt.float32, isOutput=False)
    out = nc.declare_dram_parameter("out", [ROWS, D], mybir.dt.float32, isOutput=True)

    # row index = (n*128 + p)*R + r  ->  tile n, partition p holds R
    # consecutive rows; each tile is one contiguous 4 MiB DRAM block.
    inp_t = inp[:, :].rearrange("(n p r) d -> n p (r d)", p=P, r=R)
    out_t = out[:, :].rearrange("(n p r) d -> n p (r d)", p=P, r=R)

    with TileContext(nc) as tc:
        with (
            tc.tile_pool(name="wpool", bufs=1) as wpool,
            tc.tile_pool(name="io", bufs=IO_BUFS) as io,
        ):
            wt = wpool.tile([P, D], mybir.dt.float32)
            # scalar (ACT) ring so it doesn't head-of-line-block data loads
            nc.scalar.dma_start(out=wt[:], in_=w[:, :])
            wt_b = wt[:, None, :].broadcast_to([P, R, D])
            for i in range(NTILES):
                t = io.tile([P, FREE], mybir.dt.float32)
                # loads on the SP HWDGE ring, stores on the ACT ring:
                # HWDGE is FIFO per issuing engine, so a store's sem-wait
                # must not block the next load's issue.
                nc.sync.dma_start(out=t[:], in_=inp_t[i])
                t3 = t[:].rearrange("p (r d) -> p r d", r=R)
                nc.vector.tensor_mul(out=t3, in0=t3, in1=wt_b)
                nc.scalar.dma_start(out=out_t[i], in_=t[:])
    nc.compile()
    return nc


def kernel(input, W):
    global last_exec_time_ns, _built_nc
    input = np.ascontiguousarray(np.asarray(input, dtype=np.float32))
    W = np.asarray(W, dtype=np.float32).reshape(D)

    if _built_nc is None:
        _built_nc = _build()
    nc = _built_nc

    # W replicated across the 128 partitions; the R-repeat along the free
    # dim happens on-chip via a stride-0 broadcast AP.
    w_rep = np.ascontiguousarray(np.broadcast_to(W, (P, D)))
    shards = input.reshape(NCORES, ROWS, D)
    in_maps = [{"input": shards[c], "w": w_rep} for c in range(NCORES)]

    global last_trace_dir
    trace = os.environ.get("KERNEL_TRACE", "0") == "1"
    kwargs = {}
    if trace:
        import tempfile

        last_trace_dir = tempfile.mkdtemp(prefix="diag_trace_")
        kwargs = {"trace": True, "tmpdir": last_trace_dir}
    res = run_bass_kernel_spmd(nc, in_maps, core_ids=list(range(NCORES)), **kwargs)
    last_exec_time_ns = res.exec_time_ns

    out = np.concatenate([res.results[c]["out"] for c in range(NCORES)], axis=0)
    return out
